# revision 32
# baseline (speedup 1.0000x reference)
"""BipartiteGATConv on 8 Trainium2 NeuronCores (Bass/Tile).

Strategy (dst-sharded, zero collectives):
- dst nodes partitioned across 8 cores (6250 rows each); host routes edges to
  the core owning their dst, groups them by 128-row dst block, splits each
  block's edges into lo/hi src halves (dma_gather int16 index limit), pads to
  128-edge tiles with a schedule that is identical across cores (SPMD).
- Phase 0 (on device, replicated): src table [50048, 256] bf16 rows =
  [x_src@W_src (128) | alpha_src, 0.2*alpha_src (8) | garbage pad], built by
  matmuls on host-pretransposed bf16 x_srcT (no PE transposes); dst-side
  alpha table and self term (x_dst@W_self + b_self) via host-pretransposed
  x_dstT, kept SBUF-resident.
- Edge phase: per 8-tile batch, custom dma_gather pulls 128 feats + alpha_src
  per edge; per-edge alpha_dst is expanded on chip: a rank-1 PE matmul
  broadcasts the edge dst-local indices across partitions, is_equal builds
  the transposed one-hot, and a PE contraction with the resident alpha_dst
  block lands per-edge alpha_dst in PSUM; u = alpha_src + alpha_dst is read
  straight out of PSUM. p = exp(max(u, 0.2u)) batched (leaky_relu in linear
  domain). Messages = feats * p, aggregated per dst block by
  one-hot-stationary matmuls accumulating in PSUM; softmax denominator rides
  as 4 extra rhs columns. Normalization + self-term at block finalize.
"""
import math
import numpy as np
import ml_dtypes

import concourse.bass as bass
import concourse.bacc as bacc
import concourse.tile as tile
from concourse import mybir
from concourse.masks import make_identity
from concourse.bass_utils import run_bass_kernel_spmd

N_SRC = 50000
N_DST = 50000
OUT_DIM = 128
HEADS = 4
D_HEAD = 32
NCORES = 8
DST_PER_CORE = N_DST // NCORES          # 6250
BLK = 128
N_BLK = math.ceil(DST_PER_CORE / BLK)   # 49
DST_PAD = N_BLK * BLK                   # 6272
HALF = 32768                            # lo/hi src split for int16 gather idx
SRC_PAD = 391 * 128                     # 50048 table rows
# dma_gather requires elem bytes % 256 == 0, so rows are padded to 512B
ROW = 256                               # 512B rows: 128 feat + 8 alpha bf16
P = 128
BATCH = 8                               # tiles per batch
QB = 4                                  # batches per input-DMA quad
BF = mybir.dt.bfloat16
F32 = mybir.dt.float32


def _wrap16(idx_i16):
    """[n] -> [128, n//16] int16 wrapped in 16 partitions, replicated x8."""
    n = idx_i16.shape[0]
    w = idx_i16.reshape(n // 16, 16).T  # [16, n/16]
    return np.tile(w, (8, 1))


def _preprocess(edge_src, edge_dst):
    """Route edges, build per-core tile streams + shared schedule."""
    es = np.asarray(edge_src).astype(np.int64)
    ed = np.asarray(edge_dst).astype(np.int64)
    core = ed // DST_PER_CORE
    shard = ed % DST_PER_CORE
    blk = shard // BLK
    edl = shard % BLK

    # per (core, blk, half) edge index lists
    counts = np.zeros((NCORES, N_BLK, 2), dtype=np.int64)
    lists = [[[None, None] for _ in range(N_BLK)] for _ in range(NCORES)]
    half = (es >= HALF).astype(np.int64)
    order = np.lexsort((half, blk, core))
    es_s, blk_s, edl_s, core_s, half_s = (
        es[order], blk[order], edl[order], core[order], half[order])
    key = ((core_s * N_BLK) + blk_s) * 2 + half_s
    uniq, starts = np.unique(key, return_index=True)
    starts = list(starts) + [len(key)]
    for i, k in enumerate(uniq):
        c = int(k) // (N_BLK * 2)
        b = (int(k) // 2) % N_BLK
        h = int(k) % 2
        sl = slice(starts[i], starts[i + 1])
        lists[c][b][h] = (es_s[sl], edl_s[sl])
        counts[c, b, h] = starts[i + 1] - starts[i]

    # shared tile schedule: per block, lo tiles then hi tiles (max over cores)
    t_lo = np.maximum(1, np.ceil(counts[:, :, 0] / P).astype(np.int64).max(axis=0))
    t_hi = np.ceil(counts[:, :, 1] / P).astype(np.int64).max(axis=0)
    tiles = []  # (blk, half, first_in_blk, last_in_blk)
    for b in range(N_BLK):
        n = int(t_lo[b] + t_hi[b])
        for j in range(int(t_lo[b])):
            tiles.append((b, 0, j == 0, j == n - 1))
        for j in range(int(t_hi[b])):
            tiles.append((b, 1, t_lo[b] == 0 and j == 0, j == int(t_hi[b]) - 1))
    T = len(tiles)
    while T % BATCH != 0:
        tiles.append((N_BLK - 1, 0, False, False))
        T += 1
    # padded trailing tiles belong to the last block: recompute its flags so
    # its true last tile is the final tile overall.
    lastb = N_BLK - 1
    idxs = [i for i, t in enumerate(tiles) if t[0] == lastb]
    for j, i in enumerate(idxs):
        b, h, _, _ = tiles[i]
        tiles[i] = (b, h, j == 0, j == len(idxs) - 1)

    NB = T // BATCH

    # per-batch gather runs: (half, off_tiles, ntiles) split at batch bounds
    runs = []
    for bi in range(NB):
        rr = []
        t0 = bi * BATCH
        cur_h, cur_off = tiles[t0][1], 0
        for j in range(1, BATCH):
            h = tiles[t0 + j][1]
            if h != cur_h:
                rr.append((cur_h, cur_off, j - cur_off))
                cur_h, cur_off = h, j
        rr.append((cur_h, cur_off, BATCH - cur_off))
        rr2 = []
        for (h, off, n) in rr:
            while n > 8:
                rr2.append((h, off, 8))
                off += 8
                n -= 8
            rr2.append((h, off, n))
        runs.append(rr2)

    # per-core arrays
    per_core = []
    for c in range(NCORES):
        es_tiles = np.zeros((T, P), dtype=np.int64)
        edl_tiles = np.full((T, P), -1.0, dtype=np.float32)
        fill = np.zeros(N_BLK * 2, dtype=np.int64)
        for ti, (b, h, _, _) in enumerate(tiles):
            ent = lists[c][b][h]
            if ent is None:
                if h == 1:
                    es_tiles[ti, :] = HALF  # pad hi -> local 0
                continue
            e_arr, l_arr = ent
            k = fill[b * 2 + h]
            take = min(P, max(0, len(e_arr) - k))
            if take > 0:
                es_tiles[ti, :take] = e_arr[k:k + take]
                edl_tiles[ti, :take] = l_arr[k:k + take]
            if take < P and h == 1:
                es_tiles[ti, take:] = HALF
            fill[b * 2 + h] = k + take
        loc = es_tiles.copy()
        for ti, (b, h, _, _) in enumerate(tiles):
            if h == 1:
                loc[ti] -= HALF
        esw = np.zeros((NB, P, BATCH * P // 16), dtype=np.int16)
        for bi in range(NB):
            flat = loc[bi * BATCH:(bi + 1) * BATCH].reshape(-1).astype(np.int16)
            esw[bi] = _wrap16(flat)
        edl_b = edl_tiles.reshape(NB, BATCH, P).transpose(0, 2, 1)  # [NB,P,8]
        edl_bf = edl_b.astype(ml_dtypes.bfloat16)  # 0..127 exact in bf16
        per_core.append({
            "esw": esw,
            "edl": np.ascontiguousarray(edl_bf),
        })
    return tiles, runs, NB, per_core


def _build(tiles, runs, NB, const_k=None):
    nc = bacc.Bacc("TRN2", target_bir_lowering=False, debug=False,
                   enable_asserts=True, num_devices=NCORES,
                   num_swdge_queues=4)
    T = len(tiles)

    x_srcT = nc.dram_tensor("x_srcT", [128, SRC_PAD], BF, kind="ExternalInput")
    x_dstT = nc.dram_tensor("x_dstT", [128, DST_PAD], BF, kind="ExternalInput")
    rhs_s = nc.dram_tensor("rhs_s", [128, 136], BF, kind="ExternalInput")
    rhs_d = nc.dram_tensor("rhs_d", [128, 136], BF, kind="ExternalInput")
    b_row = nc.dram_tensor("b_row", [1, 128], BF, kind="ExternalInput")
    esw = nc.dram_tensor("esw", [NB, P, BATCH * P // 16], mybir.dt.int16,
                         kind="ExternalInput")
    edl = nc.dram_tensor("edl", [NB, P, BATCH], BF, kind="ExternalInput")
    kin = nc.dram_tensor("kin", [1, 1], mybir.dt.int32, kind="ExternalInput")
    y = nc.dram_tensor("y", [DST_PAD, 128], F32, kind="ExternalOutput")

    table_lo = nc.dram_tensor("table_lo", [HALF, ROW], BF)
    table_hi = nc.dram_tensor("table_hi", [SRC_PAD - HALF, ROW], BF)

    with tile.TileContext(nc) as tc:
        with tc.tile_pool(name="const", bufs=1) as cpool, \
             tc.tile_pool(name="resident", bufs=1) as rpool:
            if const_k is None:
                kt = cpool.tile([1, 1], mybir.dt.int32)
                nc.sync.dma_start(out=kt[:], in_=kin[:, :])
                kv = nc.values_load(kt[0:1, 0:1], min_val=0, max_val=100000,
                                    skip_runtime_bounds_check=True)
            else:
                kv = const_k
            ident = cpool.tile([P, P], BF)
            make_identity(nc, ident[:])
            iotam_i = cpool.tile([P, BATCH * P], mybir.dt.int32)
            nc.gpsimd.iota(iotam_i[:], pattern=[[0, BATCH], [1, P]], base=0,
                           channel_multiplier=0)
            iota_mod = cpool.tile([P, BATCH * P], BF)
            nc.vector.tensor_copy(iota_mod[:], iotam_i[:])
            ones1 = cpool.tile([1, P], BF)
            nc.vector.memset(ones1[:], 1.0)
            rhs_s_t = cpool.tile([P, 136], BF)
            nc.sync.dma_start(out=rhs_s_t[:], in_=rhs_s[:, :])
            rhs_d_t = cpool.tile([P, 136], BF)
            nc.sync.dma_start(out=rhs_d_t[:], in_=rhs_d[:, :])
            b_t = cpool.tile([1, P], BF)
            nc.sync.dma_start(out=b_t[:], in_=b_row[:, :])

            self_all = rpool.tile([P, N_BLK * 128], F32)
            alpha_d = rpool.tile([P, N_BLK * 8], BF)

            import contextlib
            loop_ctx = (contextlib.nullcontext(0) if const_k == 1
                        else tc.For_i(0, kv))
            with loop_ctx as _i:
                # ---------------- phase 0: src table ----------------
                GRP = 8
                ngrp = math.ceil(SRC_PAD // P / GRP)
                with tc.tile_pool(name="p1x", bufs=3) as xpool, \
                     tc.tile_pool(name="p1f", bufs=3) as fpool, \
                     tc.tile_pool(name="p1ps", bufs=4, space="PSUM") as psum:
                    for g in range(ngrp):
                        j0 = g * GRP
                        jn = min(GRP, SRC_PAD // P - j0)
                        xg = xpool.tile([P, GRP * P], BF, tag="xg")
                        nc.sync.dma_start(
                            out=xg[:, 0:jn * P],
                            in_=x_srcT[:, j0 * P:(j0 + jn) * P])
                        fb = fpool.tile([P, GRP * ROW], BF, tag="fb")
                        for j4 in range(jn):
                            jj = j0 + j4
                            ps = psum.tile([P, 136], F32, tag="ps")
                            nc.tensor.matmul(ps[:],
                                             lhsT=xg[:, j4 * P:(j4 + 1) * P],
                                             rhs=rhs_s_t[:],
                                             start=True, stop=True)
                            if jj % 2 == 0:
                                nc.scalar.copy(
                                    fb[:, j4 * ROW:j4 * ROW + 136], ps[:])
                            else:
                                nc.vector.tensor_copy(
                                    fb[:, j4 * ROW:j4 * ROW + 136], ps[:])
                        r0 = j0 * P
                        fb3 = fb[:].rearrange("p (j c) -> p j c", c=ROW)
                        segs = []
                        if r0 < HALF:
                            nlo = min(jn, (HALF - r0) // P)
                            segs.append((table_lo, r0, 0, nlo))
                            if nlo < jn:
                                segs.append((table_hi, 0, nlo, jn - nlo))
                        else:
                            segs.append((table_hi, r0 - HALF, 0, jn))
                        for (tdst, rr, joff, jcnt) in segs:
                            nc.sync.dma_start(
                                out=tdst[rr:rr + jcnt * P, :].rearrange(
                                    "(j p) c -> p j c", p=P),
                                in_=fb3[:, joff:joff + jcnt, :])

                # ---------------- phase 0: dst side ----------------
                with tc.tile_pool(name="p0", bufs=3) as pool, \
                     tc.tile_pool(name="p0ps", bufs=2, space="PSUM") as psum, \
                     tc.tile_pool(name="p0ps2", bufs=2, space="PSUM") as psum2:
                    DG = 8
                    for g0 in range(0, N_BLK, DG):
                        gn = min(DG, N_BLK - g0)
                        xdg = pool.tile([P, DG * P], BF, tag="xdg")
                        nc.sync.dma_start(
                            out=xdg[:, 0:gn * P],
                            in_=x_dstT[:, g0 * P:(g0 + gn) * P])
                        for jo in range(gn):
                            j = g0 + jo
                            xT = xdg[:, jo * P:(jo + 1) * P]
                            ps_s = psum2.tile([P, P], F32, tag="ps_s")
                            nc.tensor.matmul(ps_s[:], lhsT=ones1[:],
                                             rhs=b_t[:],
                                             start=True, stop=False)
                            nc.tensor.matmul(ps_s[:], lhsT=xT,
                                             rhs=rhs_d_t[:, 0:128],
                                             start=False, stop=True)
                            ps_a = psum.tile([P, 8], F32, tag="ps_a")
                            nc.tensor.matmul(ps_a[:], lhsT=xT,
                                             rhs=rhs_d_t[:, 128:136],
                                             start=True, stop=True)
                            if j % 2 == 0:
                                nc.scalar.copy(
                                    self_all[:, j * 128:(j + 1) * 128],
                                    ps_s[:])
                            else:
                                nc.vector.tensor_copy(
                                    self_all[:, j * 128:(j + 1) * 128],
                                    ps_s[:])
                            nc.vector.tensor_copy(
                                alpha_d[:, j * 8:(j + 1) * 8], ps_a[:])

                # ---------------- edge phase ----------------
                with tc.tile_pool(name="eg", bufs=6) as gpool, \
                     tc.tile_pool(name="ei", bufs=3) as ipool, \
                     tc.tile_pool(name="es", bufs=6) as spool, \
                     tc.tile_pool(name="em", bufs=6) as mpool, \
                     tc.tile_pool(name="eu", bufs=6) as upool, \
                     tc.tile_pool(name="eo", bufs=4) as opool, \
                     tc.tile_pool(name="etp", bufs=4, space="PSUM") as psT, \
                     tc.tile_pool(name="eps", bufs=2, space="PSUM") as psA, \
                     tc.tile_pool(name="eac", bufs=2, space="PSUM") as psB:
                    qn = 0
                    acc = None
                    for bq in range(0, NB, QB):
                        qb = min(QB, NB - bq)
                        itq = ipool.tile([P, QB * BATCH * P // 16],
                                         mybir.dt.int16, tag="itq")
                        nc.sync.dma_start(
                            out=itq[:, 0:qb * BATCH * P // 16].rearrange(
                                "p (b c) -> p b c", c=BATCH * P // 16),
                            in_=esw[bq:bq + qb, :, :].rearrange(
                                "b p c -> p b c"))
                        elq = ipool.tile([P, QB * BATCH], BF, tag="elq")
                        nc.sync.dma_start(
                            out=elq[:, 0:qb * BATCH].rearrange(
                                "p (b c) -> p b c", c=BATCH),
                            in_=edl[bq:bq + qb, :, :].rearrange(
                                "b p c -> p b c"))
                        for qi in range(qb):
                            bi = bq + qi
                            it = itq[:, qi * 64:(qi + 1) * 64]
                            el = elq[:, qi * BATCH:(qi + 1) * BATCH]
                            g8 = gpool.tile([P, BATCH * ROW], BF, tag="g8")
                            g83 = g8[:].rearrange("p (t c) -> p t c", c=ROW)
                            for (h, off, ntl) in runs[bi]:
                                tsrc = table_hi if h == 1 else table_lo
                                nc.gpsimd.dma_gather(
                                    out_ap=g83[:, off:off + ntl, :],
                                    in_ap=tsrc[:, :],
                                    idxs_ap=it[:, off * 8:(off + ntl) * 8],
                                    num_idxs=ntl * P,
                                    num_idxs_reg=ntl * P,
                                    elem_size=ROW,
                                    single_packet=False,
                                    queue_num=qn % 4,
                                )
                                qn += 1
                            s8 = spool.tile([P, BATCH * P], BF, tag="s8")
                            nc.vector.tensor_tensor(
                                out=s8[:].rearrange("p (t r) -> p t r", r=P),
                                in0=iota_mod[:].rearrange(
                                    "p (t r) -> p t r", r=P),
                                in1=el[:, :, None].to_broadcast([P, BATCH, P]),
                                op=mybir.AluOpType.is_equal)
                            # transposed one-hot via PE transposes of s8,
                            # batched 4-per-PSUM-tile to amortize the copy
                            st8 = spool.tile([P, BATCH * P], BF, tag="st8")
                            for hh in range(2):
                                tp = psT.tile([P, 4 * P], F32, tag="tp")
                                for t4 in range(4):
                                    t = hh * 4 + t4
                                    nc.tensor.matmul(
                                        tp[:, t4 * P:(t4 + 1) * P],
                                        lhsT=s8[:, t * P:(t + 1) * P],
                                        rhs=ident[:],
                                        start=True, stop=True)
                                nc.scalar.copy(
                                    st8[:, hh * 4 * P:(hh + 1) * 4 * P],
                                    tp[:])
                            ade = psA.tile([P, BATCH * 8], F32, tag="ade")
                            for t in range(BATCH):
                                blk = tiles[bi * BATCH + t][0]
                                nc.tensor.matmul(
                                    ade[:, t * 8:(t + 1) * 8],
                                    lhsT=st8[:, t * P:(t + 1) * P],
                                    rhs=alpha_d[:, blk * 8:(blk + 1) * 8],
                                    start=True, stop=True)
                            # u = alpha_src (gathered) + alpha_dst (PSUM)
                            u8 = upool.tile([P, BATCH * 8], F32, tag="u8")
                            nc.vector.tensor_tensor(
                                out=u8[:].rearrange("p (t a) -> p t a", a=8),
                                in0=g83[:, :, 128:136],
                                in1=ade[:].rearrange("p (t a) -> p t a", a=8),
                                op=mybir.AluOpType.add)
                            a8 = upool.tile([P, BATCH * 4], F32, tag="a8")
                            u83 = u8[:].rearrange("p (t a) -> p t a", a=8)
                            nc.vector.tensor_tensor(
                                out=a8[:].rearrange("p (t a) -> p t a", a=4),
                                in0=u83[:, :, 0:4],
                                in1=u83[:, :, 4:8],
                                op=mybir.AluOpType.max)
                            a83 = a8[:].rearrange("p (t a) -> p t a", a=4)
                            pexp = mpool.tile([P, BATCH * P], BF, tag="pexp")
                            pexp4 = pexp[:].rearrange(
                                "p (t h d) -> p t h d", h=4, d=32)
                            nc.scalar.activation(
                                pexp4, a83[:, :, :, None].to_broadcast(
                                    [P, BATCH, 4, 32]),
                                mybir.ActivationFunctionType.Exp)
                            mp8 = mpool.tile([P, BATCH * 132], BF, tag="mp8")
                            mp83 = mp8[:].rearrange("p (t c) -> p t c", c=132)
                            nc.vector.tensor_tensor(
                                out=mp83[:, :, 0:128],
                                in0=g83[:, :, 0:128],
                                in1=pexp[:].rearrange("p (t c) -> p t c", c=P),
                                op=mybir.AluOpType.mult)
                            nc.scalar.activation(
                                mp83[:, :, 128:132], a83[:],
                                mybir.ActivationFunctionType.Exp)
                            for t in range(BATCH):
                                ti = bi * BATCH + t
                                blk, _, first, last = tiles[ti]
                                if first:
                                    acc = psB.tile([P, 132], F32, tag="acc")
                                nc.tensor.matmul(
                                    acc[:],
                                    lhsT=s8[:, t * P:(t + 1) * P],
                                    rhs=mp83[:, t, :],
                                    start=first, stop=last)
                                if last:
                                    s1 = upool.tile([P, 4], F32, tag="s1")
                                    nc.vector.tensor_scalar_add(
                                        s1[:], acc[:, 128:132], 1e-16)
                                    rv = upool.tile([P, 4], F32, tag="rv")
                                    nc.vector.reciprocal(rv[:], s1[:])
                                    ob = opool.tile([P, P], F32, tag="ob")
                                    nc.vector.tensor_tensor(
                                        out=ob[:].rearrange(
                                            "p (h d) -> p h d", h=4),
                                        in0=acc[:, 0:128].rearrange(
                                            "p (h d) -> p h d", h=4),
                                        in1=rv[:, :, None].to_broadcast(
                                            [P, 4, 32]),
                                        op=mybir.AluOpType.mult)
                                    ob2 = opool.tile([P, P], F32, tag="ob2")
                                    nc.vector.tensor_tensor(
                                        out=ob2[:], in0=ob[:],
                                        in1=self_all[:,
                                                     blk * 128:(blk + 1) * 128],
                                        op=mybir.AluOpType.add)
                                    nc.scalar.dma_start(
                                        out=y[blk * P:(blk + 1) * P, :],
                                        in_=ob2[:])
    nc.compile()
    return nc


def _host_arrays(x_src, x_dst, W_src, W_dst, att_src, att_dst, W_self, b_self):
    x_src = np.asarray(x_src, dtype=np.float32)
    x_dst = np.asarray(x_dst, dtype=np.float32)
    W_src = np.asarray(W_src, dtype=np.float32)
    W_dst = np.asarray(W_dst, dtype=np.float32)
    att_src = np.asarray(att_src, dtype=np.float32).reshape(HEADS, D_HEAD)
    att_dst = np.asarray(att_dst, dtype=np.float32).reshape(HEADS, D_HEAD)
    W_self = np.asarray(W_self, dtype=np.float32)
    b_self = np.asarray(b_self, dtype=np.float32)

    # A[f, h] one-hot-block matrices: A[h*32+d, h] = att[h, d]
    A_s = np.zeros((128, HEADS), dtype=np.float32)
    A_d = np.zeros((128, HEADS), dtype=np.float32)
    for h in range(HEADS):
        A_s[h * D_HEAD:(h + 1) * D_HEAD, h] = att_src[h]
        A_d[h * D_HEAD:(h + 1) * D_HEAD, h] = att_dst[h]
    WA_s = W_src @ A_s          # [128, 4]
    WA_d = W_dst @ A_d
    bf = ml_dtypes.bfloat16
    rhs_s = np.concatenate([W_src, WA_s, 0.2 * WA_s], axis=1).astype(bf)
    rhs_d = np.concatenate([W_self, WA_d, 0.2 * WA_d], axis=1).astype(bf)
    x_srcT = np.zeros((128, SRC_PAD), dtype=bf)
    x_srcT[:, :N_SRC] = x_src.T.astype(bf)
    return x_srcT, x_dst, rhs_s, rhs_d, b_self.reshape(1, 128).astype(bf)


def _core_x_dstT(x_dst_f, c):
    bf = ml_dtypes.bfloat16
    xdT = np.zeros((128, DST_PAD), dtype=bf)
    sh = x_dst_f[c * DST_PER_CORE:(c + 1) * DST_PER_CORE]
    xdT[:, :DST_PER_CORE] = sh.T.astype(bf)
    return xdT


def _in_maps(per_core, x_srcT, x_dst_f, rhs_s, rhs_d, b_row, k):
    ms = []
    for c in range(NCORES):
        ms.append({
            "x_srcT": x_srcT,
            "x_dstT": _core_x_dstT(x_dst_f, c),
            "rhs_s": rhs_s,
            "rhs_d": rhs_d,
            "b_row": b_row,
            "esw": per_core[c]["esw"],
            "edl": per_core[c]["edl"],
            "kin": np.array([[k]], dtype=np.int32),
        })
    return ms


_CACHE = {}


def _get_program(edge_src, edge_dst):
    key = (hash(np.asarray(edge_src).tobytes()),
           hash(np.asarray(edge_dst).tobytes()))
    if key not in _CACHE:
        tiles, runs, NB, per_core = _preprocess(edge_src, edge_dst)
        nc = _build(tiles, runs, NB)
        _CACHE[key] = (nc, per_core)
    return _CACHE[key]


def kernel(x_src, x_dst, edge_src, edge_dst, num_dst,
           W_src, W_dst, att_src, att_dst, W_self, b_self, _k=1):
    nc, per_core = _get_program(edge_src, edge_dst)
    x_srcT, x_dst_f, rhs_s, rhs_d, b_row = _host_arrays(
        x_src, x_dst, W_src, W_dst, att_src, att_dst, W_self, b_self)
    in_maps = _in_maps(per_core, x_srcT, x_dst_f, rhs_s, rhs_d, b_row, _k)
    res = run_bass_kernel_spmd(nc, in_maps, list(range(NCORES)))
    out = np.concatenate(
        [res.results[c]["y"][:DST_PER_CORE] for c in range(NCORES)], axis=0)
    return out.astype(np.float32)
